# revision 1
# baseline (speedup 1.0000x reference)
"""Trainium2 Bass kernel for nn_PrescriptionPill (segment_reduce).

Math (see reference): with xd = x (detached),
  out1[n] = x[n]@W1.T + W1_b + W2_b + loo_mean[n]@W2.T
            where loo_mean is the leave-label-out per-segment mean.
  out2[n] = LN(fc(gelu_tanh(pr)) + fc_b + pr)[seg(n)],  pr = seg_mean@proj.T + proj_b

Everything is linear in x up to the small nonlinear projection head, so:
  X1|X2|X3 = x @ [W1.T | W2.T | proj.T]        (the only big matmuls)
  out1     = X1 + b12 + A' @ X2                (A' block-diagonal per segment,
                                                scaled by 1/other_cnt, built on host
                                                from the integer labels)
  pr       = S @ X3 + proj_b                   (S = per-segment mean indicator)
then the projection head runs on [nseg, 256] only.

Sharding: data-parallel over N, aligned to label_batch segments (each of the
64 prescriptions lives on exactly one of the 8 cores); the small weights are
replicated. Rows are re-packed per core so no segment spans more than two
128-row groups; all gathers/scatters become dense 128x128 indicator matmuls.

Matmuls run as float32r (full-rate PE) accumulating into fp32 PSUM.
"""

import numpy as np
from contextlib import ExitStack

import concourse.bacc as bacc
import concourse.tile as tile
from concourse import mybir
from concourse.bass_utils import run_bass_kernel_spmd

F32 = mybir.dt.float32
F32R = mybir.dt.float32r

D = 2048
P = 256
KCH = D // 128          # 16 contraction chunks
NCORES = 8
GELU_C0 = 0.7978845608028654
GELU_C1 = 0.044715
LN_EPS = 1e-5

_PROG_CACHE = {}


# ----------------------------------------------------------------------------
# host-side planning
# ----------------------------------------------------------------------------

def _plan(label, label_batch):
    """Segment-aligned sharding + per-core row packing."""
    N = label_batch.shape[0]
    segs, seg_starts, seg_cnts = np.unique(label_batch, return_index=True,
                                           return_counts=True)
    nseg = len(segs)
    cum = np.cumsum(seg_cnts)

    bounds = [0]
    for c in range(1, NCORES):
        target = N * c / NCORES
        i = int(np.argmin(np.abs(cum - target))) + 1
        bounds.append(max(i, bounds[-1] + 1))
    bounds.append(nseg)

    cores = []
    maxslots = 0
    maxseg = 0
    for c in range(NCORES):
        s0, s1 = bounds[c], bounds[c + 1]
        q = 0
        seg_meta = []            # (slot0, cnt, orig_row0)
        for s in range(s0, s1):
            cnt = int(seg_cnts[s])
            assert cnt <= 256, "segment larger than two row groups"
            if cnt > 128 and (q % 128) + cnt > 256:
                q = ((q + 127) // 128) * 128
            seg_meta.append((q, cnt, int(seg_starts[s])))
            q += cnt
        cores.append(seg_meta)
        maxslots = max(maxslots, q)
        maxseg = max(maxseg, s1 - s0)

    NG = (maxslots + 127) // 128
    return cores, NG, maxseg


def _pack_core(seg_meta, x, label, NG, NSEG, pairs, pair_map):
    """Build this core's device input tensors."""
    NMAX = NG * 128
    NPAIR = len(pairs)
    slots_list, rows_list = [], []
    for (q0, cnt, r0) in seg_meta:
        slots_list.append(np.arange(q0, q0 + cnt))
        rows_list.append(np.arange(r0, r0 + cnt))
    slots = np.concatenate(slots_list)
    rows = np.concatenate(rows_list)

    xp = np.zeros((NMAX, D), np.float32)
    xp[slots] = x[rows]
    # xTg[g, p, k*128+n] = xp[g*128+n, k*128+p]
    xTg = np.ascontiguousarray(
        xp.reshape(NG, 128, KCH, 128).transpose(0, 3, 2, 1))

    a3T = np.zeros((NPAIR, 128, 128), np.float32)
    segind = np.zeros((128, NG, NSEG), np.float32)
    for ls, (q0, cnt, r0) in enumerate(seg_meta):
        l = label[r0:r0 + cnt]
        same = l[:, None] == l[None, :]
        other_cnt = cnt - same.sum(1)
        coef = np.where(other_cnt > 0, 1.0 / np.maximum(other_cnt, 1), 0.0)
        M = (~same) * coef[None, :].astype(np.float32)
        si = np.arange(q0, q0 + cnt)
        pidx = pair_map[si[:, None] // 128, si[None, :] // 128]
        assert (pidx >= 0).all()
        flat = (pidx * 128 + (si % 128)[:, None]) * 128 + (si % 128)[None, :]
        a3T.reshape(-1)[flat.ravel()] = M.ravel().astype(np.float32)
        segind[si % 128, si // 128, ls] = 1.0 / cnt

    # a3T device layout: [src_r, pair, tgt_r]
    a3T_dev = np.ascontiguousarray(a3T.transpose(1, 0, 2))
    return xTg, a3T_dev, segind, slots, rows


# ----------------------------------------------------------------------------
# device program
# ----------------------------------------------------------------------------

def _build_program(NG, NSEG, NPAIR, pairs):
    nc = bacc.Bacc("TRN2", target_bir_lowering=False, debug=False)
    NMAX = NG * 128

    xTg = nc.dram_tensor("xTg", [NG, 128, KCH * 128], F32R, kind="ExternalInput").ap()
    wT = nc.dram_tensor("wT", [KCH, 128, 3 * P], F32R, kind="ExternalInput").ap()
    a3T = nc.dram_tensor("a3T", [128, NPAIR, 128], F32R, kind="ExternalInput").ap()
    segind = nc.dram_tensor("segind", [128, NG, NSEG], F32R, kind="ExternalInput").ap()
    b12 = nc.dram_tensor("b12", [128, P], F32, kind="ExternalInput").ap()
    fcT = nc.dram_tensor("fcT", [128, 2, P], F32R, kind="ExternalInput").ap()
    projb = nc.dram_tensor("projb", [NSEG, P], F32, kind="ExternalInput").ap()
    fcb = nc.dram_tensor("fcb", [NSEG, P], F32, kind="ExternalInput").ap()
    lng = nc.dram_tensor("lng", [NSEG, P], F32, kind="ExternalInput").ap()
    lnb = nc.dram_tensor("lnb", [NSEG, P], F32, kind="ExternalInput").ap()
    ident = nc.dram_tensor("ident", [NSEG, NSEG], F32R, kind="ExternalInput").ap()
    y1 = nc.dram_tensor("y1", [NMAX, P], F32, kind="ExternalOutput").ap()
    y2 = nc.dram_tensor("y2", [NSEG, P], F32, kind="ExternalOutput").ap()

    # pairs with a given target group, as (src_group, pair_index)
    tgt_pairs = {t: [] for t in range(NG)}
    for pi, (sg, tg) in enumerate(pairs):
        tgt_pairs[tg].append((sg, pi))

    with tile.TileContext(nc) as tc:
        with ExitStack() as ctx:
            big = ctx.enter_context(tc.tile_pool(name="big", bufs=1))
            x3p = ctx.enter_context(tc.tile_pool(name="x3p", bufs=3))
            y1p = ctx.enter_context(tc.tile_pool(name="y1p", bufs=3))
            tail = ctx.enter_context(tc.tile_pool(name="tail", bufs=1))
            pA = ctx.enter_context(tc.tile_pool(name="pA", bufs=2, space="PSUM"))
            pB = ctx.enter_context(tc.tile_pool(name="pB", bufs=2, space="PSUM"))
            pS = ctx.enter_context(tc.tile_pool(name="pS", bufs=1, space="PSUM"))
            pT = ctx.enter_context(tc.tile_pool(name="pT", bufs=1, space="PSUM"))

            # ---- input loads ----
            w_sb = big.tile([128, KCH, 3 * P], F32R)
            for k in range(KCH):
                nc.sync.dma_start(out=w_sb[:, k, :], in_=wT[k])
            xg_sb = big.tile([128, NG, KCH * 128], F32R)
            for g in range(NG):
                nc.sync.dma_start(out=xg_sb[:, g, :], in_=xTg[g])
            b12_sb = big.tile([128, P], F32)
            nc.sync.dma_start(out=b12_sb, in_=b12)
            si_sb = big.tile([128, NG, NSEG], F32R)
            nc.sync.dma_start(out=si_sb, in_=segind)
            a3_sb = big.tile([128, NPAIR, 128], F32R)
            nc.sync.dma_start(out=a3_sb, in_=a3T)
            fcT_sb = big.tile([128, 2, P], F32R)
            nc.sync.dma_start(out=fcT_sb, in_=fcT)
            projb_sb = big.tile([NSEG, P], F32)
            nc.sync.dma_start(out=projb_sb, in_=projb)
            fcb_sb = big.tile([NSEG, P], F32)
            nc.sync.dma_start(out=fcb_sb, in_=fcb)
            lng_sb = big.tile([NSEG, P], F32)
            nc.sync.dma_start(out=lng_sb, in_=lng)
            lnb_sb = big.tile([NSEG, P], F32)
            nc.sync.dma_start(out=lnb_sb, in_=lnb)
            id_sb = big.tile([NSEG, NSEG], F32R)
            nc.sync.dma_start(out=id_sb, in_=ident)

            x1b_sb = big.tile([128, NG, P], F32)
            x2_sb = big.tile([128, NG, P], F32R)
            psS = pS.tile([NSEG, P], F32)

            def emit_b(t):
                plist = tgt_pairs[t]
                psB = pB.tile([128, P], F32, tag="pB")
                for i, (sg, pi) in enumerate(plist):
                    nc.tensor.matmul(psB, a3_sb[:, pi, :], x2_sb[:, sg, :],
                                     start=(i == 0), stop=(i == len(plist) - 1))
                y1t = y1p.tile([128, P], F32, tag="y1t")
                nc.vector.tensor_add(y1t, x1b_sb[:, t, :], psB)
                nc.sync.dma_start(out=y1[t * 128:(t + 1) * 128, :], in_=y1t)

            # ---- main loop: X123 matmuls + per-group epilogues ----
            for g in range(NG):
                psA = pA.tile([128, 3 * P], F32, tag="pA")
                for k in range(KCH):
                    lhsT = xg_sb[:, g, k * 128:(k + 1) * 128]
                    nc.tensor.matmul(psA[:, 0:512], lhsT, w_sb[:, k, 0:512],
                                     start=(k == 0), stop=(k == KCH - 1))
                    nc.tensor.matmul(psA[:, 512:768], lhsT, w_sb[:, k, 512:768],
                                     start=(k == 0), stop=(k == KCH - 1))
                nc.vector.tensor_add(x1b_sb[:, g, :], b12_sb, psA[:, 0:P])
                nc.vector.tensor_copy(x2_sb[:, g, :], psA[:, P:2 * P])
                x3t = x3p.tile([128, P], F32R, tag="x3t")
                nc.vector.tensor_copy(x3t, psA[:, 2 * P:3 * P])
                nc.tensor.matmul(psS, si_sb[:, g, :], x3t,
                                 start=(g == 0), stop=(g == NG - 1))
                if g >= 1:
                    emit_b(g - 1)
            emit_b(NG - 1)

            # ---- projection head on [NSEG, 256] ----
            pr = tail.tile([NSEG, P], F32, tag="pr")
            nc.vector.tensor_add(pr, projb_sb, psS)
            t_x2 = tail.tile([NSEG, P], F32, tag="t_x2")
            nc.vector.tensor_mul(t_x2, pr, pr)
            t_x3 = tail.tile([NSEG, P], F32, tag="t_x3")
            nc.vector.tensor_mul(t_x3, t_x2, pr)
            t_u = tail.tile([NSEG, P], F32, tag="t_u")
            nc.vector.scalar_tensor_tensor(t_u, t_x3, GELU_C1, pr,
                                           op0=mybir.AluOpType.mult,
                                           op1=mybir.AluOpType.add)
            t_t = tail.tile([NSEG, P], F32, tag="t_t")
            nc.scalar.activation(t_t, t_u, mybir.ActivationFunctionType.Tanh,
                                 scale=GELU_C0)
            t_ph = tail.tile([NSEG, P], F32, tag="t_ph")
            nc.scalar.mul(t_ph, pr, 0.5)
            t_q = tail.tile([NSEG, P], F32, tag="t_q")
            nc.vector.tensor_mul(t_q, t_ph, t_t)
            t_g = tail.tile([NSEG, P], F32R, tag="t_g")
            nc.vector.tensor_add(t_g, t_ph, t_q)

            gT = tail.tile([128, 2, NSEG], F32R, tag="gT")
            for c in range(2):
                ptr = pT.tile([128, NSEG], F32R, tag="ptr")
                nc.tensor.transpose(ptr, t_g[:, c * 128:(c + 1) * 128], id_sb)
                nc.vector.tensor_copy(gT[:, c, :], ptr)

            psF = pB.tile([128, P], F32, tag="pB")
            for c in range(2):
                nc.tensor.matmul(psF[0:NSEG, :], gT[:, c, :], fcT_sb[:, c, :],
                                 start=(c == 0), stop=(c == 1))
            t_h0 = tail.tile([NSEG, P], F32, tag="t_h0")
            nc.vector.tensor_add(t_h0, fcb_sb, psF[0:NSEG, :])
            t_h = tail.tile([NSEG, P], F32, tag="t_h")
            nc.vector.tensor_add(t_h, t_h0, pr)

            stats = tail.tile([NSEG, 6], F32, tag="stats")
            nc.vector.bn_stats(out=stats, in_=t_h)
            mv = tail.tile([NSEG, 2], F32, tag="mv")
            nc.vector.bn_aggr(out=mv, in_=stats)
            epst = tail.tile([NSEG, 1], F32, tag="epst")
            nc.vector.memset(epst, LN_EPS)
            sd = tail.tile([NSEG, 1], F32, tag="sd")
            nc.scalar.activation(sd, mv[:, 1:2], mybir.ActivationFunctionType.Sqrt,
                                 bias=epst)
            rstd = tail.tile([NSEG, 1], F32, tag="rstd")
            nc.vector.reciprocal(rstd, sd)
            t_d = tail.tile([NSEG, P], F32, tag="t_d")
            nc.vector.tensor_scalar(t_d, t_h, mv[:, 0:1], None,
                                    op0=mybir.AluOpType.subtract)
            t_dn = tail.tile([NSEG, P], F32, tag="t_dn")
            nc.vector.tensor_scalar(t_dn, t_d, rstd, None,
                                    op0=mybir.AluOpType.mult)
            t_y2g = tail.tile([NSEG, P], F32, tag="t_y2g")
            nc.vector.tensor_mul(t_y2g, t_dn, lng_sb)
            t_y2 = tail.tile([NSEG, P], F32, tag="t_y2")
            nc.vector.tensor_add(t_y2, t_y2g, lnb_sb)
            nc.sync.dma_start(out=y2, in_=t_y2)

    nc.compile()
    return nc


# ----------------------------------------------------------------------------
# entry point
# ----------------------------------------------------------------------------

def kernel(x, label, label_batch, W1_w, W1_b, W2_w, W2_b,
           proj_w, proj_b, fc_w, fc_b, ln_g, ln_b):
    x = np.asarray(x, np.float32)
    label = np.asarray(label)
    label_batch = np.asarray(label_batch)
    N = x.shape[0]

    cores, NG, NSEG = _plan(label, label_batch)
    pairs = ([(g, g) for g in range(NG)]
             + [(g, g + 1) for g in range(NG - 1)]
             + [(g + 1, g) for g in range(NG - 1)])
    NPAIR = len(pairs)
    pair_map = -np.ones((NG, NG), np.int64)
    for pi, (sg, tg) in enumerate(pairs):
        pair_map[sg, tg] = pi

    key = (NG, NSEG, NPAIR)
    if key not in _PROG_CACHE:
        _PROG_CACHE[key] = _build_program(NG, NSEG, NPAIR, pairs)
    nc = _PROG_CACHE[key]

    # replicated weights
    W123T = np.ascontiguousarray(
        np.concatenate([np.asarray(W1_w).T, np.asarray(W2_w).T,
                        np.asarray(proj_w).T], axis=1).astype(np.float32))
    wT_dev = np.ascontiguousarray(W123T.reshape(KCH, 128, 3 * P))
    b12_dev = np.ascontiguousarray(
        np.broadcast_to((np.asarray(W1_b) + np.asarray(W2_b)).astype(np.float32),
                        (128, P)))
    fcT_dev = np.ascontiguousarray(
        np.asarray(fc_w).T.astype(np.float32).reshape(2, 128, P).transpose(1, 0, 2))

    def rep(v):
        return np.ascontiguousarray(
            np.broadcast_to(np.asarray(v).astype(np.float32), (NSEG, P)))

    projb_dev, fcb_dev = rep(proj_b), rep(fc_b)
    lng_dev, lnb_dev = rep(ln_g), rep(ln_b)
    ident_dev = np.eye(NSEG, dtype=np.float32)

    in_maps = []
    packs = []
    for c in range(NCORES):
        xTg, a3T_dev, segind_dev, slots, rows = _pack_core(
            cores[c], x, label, NG, NSEG, pairs, pair_map)
        packs.append((slots, rows, cores[c]))
        in_maps.append({
            "xTg": xTg, "wT": wT_dev, "a3T": a3T_dev, "segind": segind_dev,
            "b12": b12_dev, "fcT": fcT_dev, "projb": projb_dev, "fcb": fcb_dev,
            "lng": lng_dev, "lnb": lnb_dev, "ident": ident_dev,
        })

    res = run_bass_kernel_spmd(nc, in_maps, list(range(NCORES)))

    out1 = np.zeros((N, P), np.float32)
    out2 = np.zeros((N, P), np.float32)
    for c in range(NCORES):
        slots, rows, seg_meta = packs[c]
        out1[rows] = res.results[c]["y1"][slots]
        y2c = res.results[c]["y2"]
        for ls, (q0, cnt, r0) in enumerate(seg_meta):
            out2[r0:r0 + cnt] = y2c[ls]
    return out1, out2


# revision 5
# speedup vs baseline: 1.6343x; 1.6343x over previous
"""Trainium2 Bass kernel for nn_PrescriptionPill (segment_reduce).

Math (see reference): with xd = x (detached),
  out1[n] = x[n]@W1.T + W1_b + W2_b + loo_mean[n]@W2.T
            where loo_mean is the leave-label-out per-segment mean.
  out2[n] = LN(fc(gelu_tanh(pr)) + fc_b + pr)[seg(n)],  pr = seg_mean@proj.T + proj_b

Everything is linear in x up to the small nonlinear projection head, so:
  X1|X2|X3 = x @ [W1.T | W2.T | proj.T]        (the only big matmuls)
  out1     = X1 + b12 + A' @ X2                (A' block-diagonal per segment,
                                                scaled by 1/other_cnt, built on host
                                                from the integer labels)
  pr       = S @ X3 + proj_b                   (S = per-segment mean indicator)
then the projection head runs on [nseg, 256] only.

Sharding: data-parallel over N, aligned to label_batch segments (each of the
64 prescriptions lives on exactly one of the 8 cores); the small weights are
replicated. Rows are re-packed per core so no segment spans more than two
128-row groups; all gathers/scatters become dense 128x128 indicator matmuls.

Matmuls run as float32r (full-rate PE) accumulating into fp32 PSUM.
"""

import numpy as np
import ml_dtypes
from contextlib import ExitStack

import concourse.bacc as bacc
import concourse.tile as tile
from concourse import mybir
from concourse.bass_utils import run_bass_kernel_spmd

F32 = mybir.dt.float32
F32R = mybir.dt.float32r
BF16 = mybir.dt.bfloat16

# dtype of the two big streamed operands (x and the stacked weights).
# bf16 halves their DMA traffic; PE streams 1 col/cycle either way.
BIG_DT = BF16

D = 2048
P = 256
KCH = D // 128          # 16 contraction chunks
NCORES = 8
GELU_C0 = 0.7978845608028654
GELU_C1 = 0.044715
LN_EPS = 1e-5

_PROG_CACHE = {}


# ----------------------------------------------------------------------------
# host-side planning
# ----------------------------------------------------------------------------

def _plan(label, label_batch):
    """Segment-aligned sharding + per-core row packing."""
    N = label_batch.shape[0]
    segs, seg_starts, seg_cnts = np.unique(label_batch, return_index=True,
                                           return_counts=True)
    nseg = len(segs)
    cum = np.cumsum(seg_cnts)

    bounds = [0]
    for c in range(1, NCORES):
        target = N * c / NCORES
        i = int(np.argmin(np.abs(cum - target))) + 1
        bounds.append(max(i, bounds[-1] + 1))
    bounds.append(nseg)

    cores = []
    maxslots = 0
    maxseg = 0
    for c in range(NCORES):
        s0, s1 = bounds[c], bounds[c + 1]
        q = 0
        seg_meta = []            # (slot0, cnt, orig_row0)
        for s in range(s0, s1):
            cnt = int(seg_cnts[s])
            assert cnt <= 256, "segment larger than two row groups"
            if cnt > 128 and (q % 128) + cnt > 256:
                q = ((q + 127) // 128) * 128
            seg_meta.append((q, cnt, int(seg_starts[s])))
            q += cnt
        cores.append(seg_meta)
        maxslots = max(maxslots, q)
        maxseg = max(maxseg, s1 - s0)

    NG = (maxslots + 127) // 128
    return cores, NG, maxseg


def _pack_core(seg_meta, x, label, NG, NSEG, pairs, pair_map):
    """Build this core's device input tensors."""
    NMAX = NG * 128
    NPAIR = len(pairs)
    slots_list, rows_list = [], []
    for (q0, cnt, r0) in seg_meta:
        slots_list.append(np.arange(q0, q0 + cnt))
        rows_list.append(np.arange(r0, r0 + cnt))
    slots = np.concatenate(slots_list)
    rows = np.concatenate(rows_list)

    xp = np.zeros((NMAX, D), np.float32)
    xp[slots] = x[rows]
    # xTg[g, p, k*128+n] = xp[g*128+n, k*128+p]
    xTg = np.ascontiguousarray(
        xp.reshape(NG, 128, KCH, 128).transpose(0, 3, 2, 1))

    a3T = np.zeros((NPAIR, 128, 128), np.float32)
    segind = np.zeros((128, NG, NSEG), np.float32)
    for ls, (q0, cnt, r0) in enumerate(seg_meta):
        l = label[r0:r0 + cnt]
        same = l[:, None] == l[None, :]
        other_cnt = cnt - same.sum(1)
        coef = np.where(other_cnt > 0, 1.0 / np.maximum(other_cnt, 1), 0.0)
        M = (~same) * coef[None, :].astype(np.float32)
        si = np.arange(q0, q0 + cnt)
        pidx = pair_map[si[:, None] // 128, si[None, :] // 128]
        assert (pidx >= 0).all()
        flat = (pidx * 128 + (si % 128)[:, None]) * 128 + (si % 128)[None, :]
        a3T.reshape(-1)[flat.ravel()] = M.ravel().astype(np.float32)
        segind[si % 128, si // 128, ls] = 1.0 / cnt

    # a3T device layout: [src_r, pair, tgt_r]
    a3T_dev = np.ascontiguousarray(a3T.transpose(1, 0, 2))
    return xTg, a3T_dev, segind, slots, rows


# ----------------------------------------------------------------------------
# device program
# ----------------------------------------------------------------------------

def _build_program(NG, NSEG, NPAIR, pairs):
    nc = bacc.Bacc("TRN2", target_bir_lowering=False, debug=False)
    NMAX = NG * 128

    xTg = nc.dram_tensor("xTg", [NG, 128, KCH * 128], BIG_DT, kind="ExternalInput").ap()
    wT = nc.dram_tensor("wT", [KCH, 128, 3 * P], BIG_DT, kind="ExternalInput").ap()
    a3T = nc.dram_tensor("a3T", [128, NPAIR, 128], F32R, kind="ExternalInput").ap()
    segind = nc.dram_tensor("segind", [128, NG, NSEG], F32R, kind="ExternalInput").ap()
    b12 = nc.dram_tensor("b12", [128, P], F32, kind="ExternalInput").ap()
    fcT = nc.dram_tensor("fcT", [128, 2, P], F32R, kind="ExternalInput").ap()
    projb = nc.dram_tensor("projb", [NSEG, P], F32, kind="ExternalInput").ap()
    fcb = nc.dram_tensor("fcb", [NSEG, P], F32, kind="ExternalInput").ap()
    lng = nc.dram_tensor("lng", [NSEG, P], F32, kind="ExternalInput").ap()
    lnb = nc.dram_tensor("lnb", [NSEG, P], F32, kind="ExternalInput").ap()
    ident = nc.dram_tensor("ident", [NSEG, NSEG], F32R, kind="ExternalInput").ap()
    y1 = nc.dram_tensor("y1", [NMAX, P], F32, kind="ExternalOutput").ap()
    y2 = nc.dram_tensor("y2", [NSEG, P], F32, kind="ExternalOutput").ap()

    # pairs with a given target group, as (src_group, pair_index)
    tgt_pairs = {t: [] for t in range(NG)}
    for pi, (sg, tg) in enumerate(pairs):
        tgt_pairs[tg].append((sg, pi))

    with tile.TileContext(nc) as tc:
        with ExitStack() as ctx:
            big = ctx.enter_context(tc.tile_pool(name="big", bufs=1))
            x3p = ctx.enter_context(tc.tile_pool(name="x3p", bufs=3))
            y1p = ctx.enter_context(tc.tile_pool(name="y1p", bufs=3))
            tail = ctx.enter_context(tc.tile_pool(name="tail", bufs=1))
            pA = ctx.enter_context(tc.tile_pool(name="pA", bufs=2, space="PSUM"))
            pB = ctx.enter_context(tc.tile_pool(name="pB", bufs=2, space="PSUM"))
            pS = ctx.enter_context(tc.tile_pool(name="pS", bufs=1, space="PSUM"))
            pT = ctx.enter_context(tc.tile_pool(name="pT", bufs=1, space="PSUM"))

            # ---- input loads ----
            # Order matters: the DMA stream is ~bandwidth-serial, and PE's
            # first group needs xg0 + W chunks in k order. Everything later
            # (xg1.., a3, tail constants) follows.
            xg_sb = big.tile([128, NG, KCH * 128], BIG_DT)
            nc.sync.dma_start(out=xg_sb[:, 0, :], in_=xTg[0])
            w_sb = big.tile([128, KCH, 3 * P], BIG_DT)
            for k in range(KCH):
                nc.sync.dma_start(out=w_sb[:, k, :], in_=wT[k])
            b12_sb = big.tile([128, P], F32)
            nc.sync.dma_start(out=b12_sb, in_=b12)
            si_sb = big.tile([128, NG, NSEG], F32R)
            nc.sync.dma_start(out=si_sb, in_=segind)
            fcT_sb = big.tile([128, 2, P], F32R)
            nc.sync.dma_start(out=fcT_sb, in_=fcT)
            projb_sb = big.tile([NSEG, P], F32)
            nc.sync.dma_start(out=projb_sb, in_=projb)
            fcb_sb = big.tile([NSEG, P], F32)
            nc.sync.dma_start(out=fcb_sb, in_=fcb)
            lng_sb = big.tile([NSEG, P], F32)
            nc.sync.dma_start(out=lng_sb, in_=lng)
            lnb_sb = big.tile([NSEG, P], F32)
            nc.sync.dma_start(out=lnb_sb, in_=lnb)
            id_sb = big.tile([NSEG, NSEG], F32R)
            nc.sync.dma_start(out=id_sb, in_=ident)
            nc.sync.dma_start(out=xg_sb[:, 1, :], in_=xTg[1])
            a3_sb = big.tile([128, NPAIR, 128], F32R)
            nc.sync.dma_start(out=a3_sb, in_=a3T)
            for g in range(2, NG):
                nc.sync.dma_start(out=xg_sb[:, g, :], in_=xTg[g])

            x1b_sb = big.tile([128, NG, P], F32)
            x2_sb = big.tile([128, NG, P], F32R)
            psS = pS.tile([NSEG, P], F32)

            # Warm the ACT function tables (Tanh/Copy/Sqrt) during the DMA
            # phase so LoadActFuncSet is off the critical path of the tail.
            warm = tail.tile([1, 1], F32, tag="warm")
            nc.vector.memset(warm, 0.0)
            warm2 = tail.tile([1, 1], F32, tag="warm2")
            nc.scalar.activation(warm2, warm, mybir.ActivationFunctionType.Tanh,
                                 scale=1.0)
            nc.scalar.activation(warm2, warm, mybir.ActivationFunctionType.Sqrt,
                                 bias=warm)

            def emit_b(t):
                plist = tgt_pairs[t]
                psB = pB.tile([128, P], F32, tag="pB")
                for i, (sg, pi) in enumerate(plist):
                    nc.tensor.matmul(psB, a3_sb[:, pi, :], x2_sb[:, sg, :],
                                     start=(i == 0), stop=(i == len(plist) - 1))
                y1t = y1p.tile([128, P], F32, tag="y1t")
                nc.vector.tensor_add(y1t, x1b_sb[:, t, :], psB)
                nc.sync.dma_start(out=y1[t * 128:(t + 1) * 128, :], in_=y1t)

            # ---- main loop: X123 matmuls + per-group epilogues ----
            for g in range(NG):
                psA = pA.tile([128, 3 * P], F32, tag="pA")
                for k in range(KCH):
                    lhsT = xg_sb[:, g, k * 128:(k + 1) * 128]
                    nc.tensor.matmul(psA[:, 0:512], lhsT, w_sb[:, k, 0:512],
                                     start=(k == 0), stop=(k == KCH - 1))
                    nc.tensor.matmul(psA[:, 512:768], lhsT, w_sb[:, k, 512:768],
                                     start=(k == 0), stop=(k == KCH - 1))
                nc.vector.tensor_add(x1b_sb[:, g, :], b12_sb, psA[:, 0:P])
                nc.vector.tensor_copy(x2_sb[:, g, :], psA[:, P:2 * P])
                x3t = x3p.tile([128, P], F32R, tag="x3t")
                nc.vector.tensor_copy(x3t, psA[:, 2 * P:3 * P])
                nc.tensor.matmul(psS, si_sb[:, g, :], x3t,
                                 start=(g == 0), stop=(g == NG - 1))
                if g >= 1:
                    emit_b(g - 1)

            # ---- projection head on [NSEG, 256] ----
            # DVE/ACT part overlaps with the remaining A'/output matmuls.
            pr = tail.tile([NSEG, P], F32, tag="pr")
            nc.vector.tensor_add(pr, projb_sb, psS)
            t_x2 = tail.tile([NSEG, P], F32, tag="t_x2")
            nc.vector.tensor_mul(t_x2, pr, pr)
            t_x3 = tail.tile([NSEG, P], F32, tag="t_x3")
            nc.vector.tensor_mul(t_x3, t_x2, pr)
            t_u = tail.tile([NSEG, P], F32, tag="t_u")
            nc.vector.scalar_tensor_tensor(t_u, t_x3, GELU_C1, pr,
                                           op0=mybir.AluOpType.mult,
                                           op1=mybir.AluOpType.add)
            t_t = tail.tile([NSEG, P], F32, tag="t_t")
            nc.scalar.activation(t_t, t_u, mybir.ActivationFunctionType.Tanh,
                                 scale=GELU_C0)
            t_ph = tail.tile([NSEG, P], F32, tag="t_ph")
            nc.vector.tensor_scalar_mul(t_ph, pr, 0.5)
            t_q = tail.tile([NSEG, P], F32, tag="t_q")
            nc.vector.tensor_mul(t_q, t_ph, t_t)
            t_g = tail.tile([NSEG, P], F32R, tag="t_g")
            nc.vector.tensor_add(t_g, t_ph, t_q)

            emit_b(NG - 1)

            gT = tail.tile([128, 2, NSEG], F32R, tag="gT")
            for c in range(2):
                ptr = pT.tile([128, NSEG], F32R, tag="ptr")
                nc.tensor.transpose(ptr, t_g[:, c * 128:(c + 1) * 128], id_sb)
                nc.vector.tensor_copy(gT[:, c, :], ptr)

            psF = pB.tile([128, P], F32, tag="pB")
            for c in range(2):
                nc.tensor.matmul(psF[0:NSEG, :], gT[:, c, :], fcT_sb[:, c, :],
                                 start=(c == 0), stop=(c == 1))
            t_h0 = tail.tile([NSEG, P], F32, tag="t_h0")
            nc.vector.tensor_add(t_h0, fcb_sb, psF[0:NSEG, :])
            t_h = tail.tile([NSEG, P], F32, tag="t_h")
            nc.vector.tensor_add(t_h, t_h0, pr)

            stats = tail.tile([NSEG, 6], F32, tag="stats")
            nc.vector.bn_stats(out=stats, in_=t_h)
            mv = tail.tile([NSEG, 2], F32, tag="mv")
            nc.vector.bn_aggr(out=mv, in_=stats)
            epst = tail.tile([NSEG, 1], F32, tag="epst")
            nc.vector.memset(epst, LN_EPS)
            sd = tail.tile([NSEG, 1], F32, tag="sd")
            nc.scalar.activation(sd, mv[:, 1:2], mybir.ActivationFunctionType.Sqrt,
                                 bias=epst)
            rstd = tail.tile([NSEG, 1], F32, tag="rstd")
            nc.vector.reciprocal(rstd, sd)
            t_d = tail.tile([NSEG, P], F32, tag="t_d")
            nc.vector.tensor_scalar(t_d, t_h, mv[:, 0:1], None,
                                    op0=mybir.AluOpType.subtract)
            t_dn = tail.tile([NSEG, P], F32, tag="t_dn")
            nc.vector.tensor_scalar(t_dn, t_d, rstd, None,
                                    op0=mybir.AluOpType.mult)
            t_y2g = tail.tile([NSEG, P], F32, tag="t_y2g")
            nc.vector.tensor_mul(t_y2g, t_dn, lng_sb)
            t_y2 = tail.tile([NSEG, P], F32, tag="t_y2")
            nc.vector.tensor_add(t_y2, t_y2g, lnb_sb)
            nc.sync.dma_start(out=y2, in_=t_y2)

    nc.compile()
    return nc


# ----------------------------------------------------------------------------
# entry point
# ----------------------------------------------------------------------------

def kernel(x, label, label_batch, W1_w, W1_b, W2_w, W2_b,
           proj_w, proj_b, fc_w, fc_b, ln_g, ln_b):
    x = np.asarray(x, np.float32)
    label = np.asarray(label)
    label_batch = np.asarray(label_batch)
    N = x.shape[0]

    cores, NG, NSEG = _plan(label, label_batch)
    pairs = ([(g, g) for g in range(NG)]
             + [(g, g + 1) for g in range(NG - 1)]
             + [(g + 1, g) for g in range(NG - 1)])
    NPAIR = len(pairs)
    pair_map = -np.ones((NG, NG), np.int64)
    for pi, (sg, tg) in enumerate(pairs):
        pair_map[sg, tg] = pi

    key = (NG, NSEG, NPAIR)
    if key not in _PROG_CACHE:
        _PROG_CACHE[key] = _build_program(NG, NSEG, NPAIR, pairs)
    nc = _PROG_CACHE[key]

    # replicated weights
    W123T = np.ascontiguousarray(
        np.concatenate([np.asarray(W1_w).T, np.asarray(W2_w).T,
                        np.asarray(proj_w).T], axis=1).astype(np.float32))
    wT_dev = np.ascontiguousarray(W123T.reshape(KCH, 128, 3 * P))
    if BIG_DT == BF16:
        wT_dev = wT_dev.astype(ml_dtypes.bfloat16)
    b12_dev = np.ascontiguousarray(
        np.broadcast_to((np.asarray(W1_b) + np.asarray(W2_b)).astype(np.float32),
                        (128, P)))
    fcT_dev = np.ascontiguousarray(
        np.asarray(fc_w).T.astype(np.float32).reshape(2, 128, P).transpose(1, 0, 2))

    def rep(v):
        return np.ascontiguousarray(
            np.broadcast_to(np.asarray(v).astype(np.float32), (NSEG, P)))

    projb_dev, fcb_dev = rep(proj_b), rep(fc_b)
    lng_dev, lnb_dev = rep(ln_g), rep(ln_b)
    ident_dev = np.eye(NSEG, dtype=np.float32)

    in_maps = []
    packs = []
    for c in range(NCORES):
        xTg, a3T_dev, segind_dev, slots, rows = _pack_core(
            cores[c], x, label, NG, NSEG, pairs, pair_map)
        if BIG_DT == BF16:
            xTg = xTg.astype(ml_dtypes.bfloat16)
        packs.append((slots, rows, cores[c]))
        in_maps.append({
            "xTg": xTg, "wT": wT_dev, "a3T": a3T_dev, "segind": segind_dev,
            "b12": b12_dev, "fcT": fcT_dev, "projb": projb_dev, "fcb": fcb_dev,
            "lng": lng_dev, "lnb": lnb_dev, "ident": ident_dev,
        })

    res = run_bass_kernel_spmd(nc, in_maps, list(range(NCORES)))

    out1 = np.zeros((N, P), np.float32)
    out2 = np.zeros((N, P), np.float32)
    for c in range(NCORES):
        slots, rows, seg_meta = packs[c]
        out1[rows] = res.results[c]["y1"][slots]
        y2c = res.results[c]["y2"]
        for ls, (q0, cnt, r0) in enumerate(seg_meta):
            out2[r0:r0 + cnt] = y2c[ls]
    return out1, out2


# revision 8
# speedup vs baseline: 1.7804x; 1.0894x over previous
"""Trainium2 Bass kernel for nn_PrescriptionPill (segment_reduce).

Math (see reference): with xd = x (detached),
  out1[n] = x[n]@W1.T + W1_b + W2_b + loo_mean[n]@W2.T
            where loo_mean is the leave-label-out per-segment mean.
  out2[n] = LN(fc(gelu_tanh(pr)) + fc_b + pr)[seg(n)],  pr = seg_mean@proj.T + proj_b

Everything is linear in x up to the small nonlinear projection head, so:
  X1|X2|X3 = x @ [W1.T | W2.T | proj.T]        (the only big matmuls)
  out1     = X1 + b12 + A' @ X2                (A' block-diagonal per segment,
                                                scaled by 1/other_cnt, built on host
                                                from the integer labels)
  pr       = S @ X3 + proj_b                   (S = per-segment mean indicator)
then the projection head runs on [nseg, 256] only.

Sharding: data-parallel over N, aligned to label_batch segments (each of the
64 prescriptions lives on exactly one of the 8 cores); the small weights are
replicated. Rows are re-packed per core so no segment spans more than two
128-row groups; all gathers/scatters become dense 128x128 indicator matmuls.

Matmuls run as float32r (full-rate PE) accumulating into fp32 PSUM.
"""

import numpy as np
import ml_dtypes
from contextlib import ExitStack

import concourse.bacc as bacc
import concourse.tile as tile
from concourse import mybir
from concourse.bass_utils import run_bass_kernel_spmd

F32 = mybir.dt.float32
F32R = mybir.dt.float32r
BF16 = mybir.dt.bfloat16

# dtype of the two big streamed operands (x and the stacked weights).
# bf16 halves their DMA traffic; PE streams 1 col/cycle either way.
BIG_DT = BF16

D = 2048
P = 256
KCH = D // 128          # 16 contraction chunks
NCORES = 8
GELU_C0 = 0.7978845608028654
GELU_C1 = 0.044715
LN_EPS = 1e-5

_PROG_CACHE = {}


# ----------------------------------------------------------------------------
# host-side planning
# ----------------------------------------------------------------------------

def _plan(label, label_batch):
    """Segment-aligned sharding + per-core row packing."""
    N = label_batch.shape[0]
    segs, seg_starts, seg_cnts = np.unique(label_batch, return_index=True,
                                           return_counts=True)
    nseg = len(segs)
    cum = np.cumsum(seg_cnts)

    bounds = [0]
    for c in range(1, NCORES):
        target = N * c / NCORES
        i = int(np.argmin(np.abs(cum - target))) + 1
        bounds.append(max(i, bounds[-1] + 1))
    bounds.append(nseg)

    cores = []
    maxslots = 0
    maxseg = 0
    for c in range(NCORES):
        s0, s1 = bounds[c], bounds[c + 1]
        q = 0
        seg_meta = []            # (slot0, cnt, orig_row0)
        for s in range(s0, s1):
            cnt = int(seg_cnts[s])
            assert cnt <= 256, "segment larger than two row groups"
            if cnt > 128 and (q % 128) + cnt > 256:
                q = ((q + 127) // 128) * 128
            seg_meta.append((q, cnt, int(seg_starts[s])))
            q += cnt
        cores.append(seg_meta)
        maxslots = max(maxslots, q)
        maxseg = max(maxseg, s1 - s0)

    NG = (maxslots + 127) // 128
    return cores, NG, maxseg


def _pack_core(seg_meta, x, label, NG, NSEG, pairs, pair_map):
    """Build this core's device input tensors."""
    NMAX = NG * 128
    NPAIR = len(pairs)
    slots_list, rows_list = [], []
    for (q0, cnt, r0) in seg_meta:
        slots_list.append(np.arange(q0, q0 + cnt))
        rows_list.append(np.arange(r0, r0 + cnt))
    slots = np.concatenate(slots_list)
    rows = np.concatenate(rows_list)

    xp = np.zeros((NMAX, D), np.float32)
    xp[slots] = x[rows]
    # xTg[g, p, k*128+n] = xp[g*128+n, k*128+p]
    xTg = np.ascontiguousarray(
        xp.reshape(NG, 128, KCH, 128).transpose(0, 3, 2, 1))

    a3T = np.zeros((NPAIR, 128, 128), np.float32)
    segind = np.zeros((128, NG, NSEG), np.float32)
    for ls, (q0, cnt, r0) in enumerate(seg_meta):
        l = label[r0:r0 + cnt]
        same = l[:, None] == l[None, :]
        other_cnt = cnt - same.sum(1)
        coef = np.where(other_cnt > 0, 1.0 / np.maximum(other_cnt, 1), 0.0)
        M = (~same) * coef[None, :].astype(np.float32)
        si = np.arange(q0, q0 + cnt)
        pidx = pair_map[si[:, None] // 128, si[None, :] // 128]
        assert (pidx >= 0).all()
        flat = (pidx * 128 + (si % 128)[:, None]) * 128 + (si % 128)[None, :]
        a3T.reshape(-1)[flat.ravel()] = M.ravel().astype(np.float32)
        segind[si % 128, si // 128, ls] = 1.0 / cnt

    # a3T device layout: [src_r, pair, tgt_r]
    a3T_dev = np.ascontiguousarray(a3T.transpose(1, 0, 2))
    return xTg, a3T_dev, segind, slots, rows


# ----------------------------------------------------------------------------
# device program
# ----------------------------------------------------------------------------

def _build_program(NG, NSEG, NPAIR, pairs):
    nc = bacc.Bacc("TRN2", target_bir_lowering=False, debug=False)
    NMAX = NG * 128

    xTg = nc.dram_tensor("xTg", [NG, 128, KCH * 128], BIG_DT, kind="ExternalInput").ap()
    wT = nc.dram_tensor("wT", [KCH, 128, 3 * P], BIG_DT, kind="ExternalInput").ap()
    a3T = nc.dram_tensor("a3T", [128, NPAIR, 128], BIG_DT, kind="ExternalInput").ap()
    segind = nc.dram_tensor("segind", [128, NG, NSEG], F32R, kind="ExternalInput").ap()
    b12 = nc.dram_tensor("b12", [128, P], F32, kind="ExternalInput").ap()
    fcT = nc.dram_tensor("fcT", [128, 2, P], F32R, kind="ExternalInput").ap()
    projb = nc.dram_tensor("projb", [NSEG, P], F32, kind="ExternalInput").ap()
    fcb = nc.dram_tensor("fcb", [NSEG, P], F32, kind="ExternalInput").ap()
    lng = nc.dram_tensor("lng", [NSEG, P], F32, kind="ExternalInput").ap()
    lnb = nc.dram_tensor("lnb", [NSEG, P], F32, kind="ExternalInput").ap()
    ident = nc.dram_tensor("ident", [NSEG, NSEG], F32R, kind="ExternalInput").ap()
    y1 = nc.dram_tensor("y1", [NMAX, P], F32, kind="ExternalOutput").ap()
    y2 = nc.dram_tensor("y2", [NSEG, P], F32, kind="ExternalOutput").ap()

    # pairs with a given target group, as (src_group, pair_index)
    tgt_pairs = {t: [] for t in range(NG)}
    for pi, (sg, tg) in enumerate(pairs):
        tgt_pairs[tg].append((sg, pi))

    with tile.TileContext(nc) as tc:
        with ExitStack() as ctx:
            big = ctx.enter_context(tc.tile_pool(name="big", bufs=1))
            x3p = ctx.enter_context(tc.tile_pool(name="x3p", bufs=3))
            y1p = ctx.enter_context(tc.tile_pool(name="y1p", bufs=3))
            tail = ctx.enter_context(tc.tile_pool(name="tail", bufs=1))
            pA = ctx.enter_context(tc.tile_pool(name="pA", bufs=2, space="PSUM"))
            pB = ctx.enter_context(tc.tile_pool(name="pB", bufs=2, space="PSUM"))
            pS = ctx.enter_context(tc.tile_pool(name="pS", bufs=1, space="PSUM"))
            pT = ctx.enter_context(tc.tile_pool(name="pT", bufs=1, space="PSUM"))

            # ---- input loads ----
            # Order matters: the DMA stream is ~bandwidth-serial, and PE's
            # first group needs xg0 + W chunks in k order. Everything later
            # (xg1.., a3, tail constants) follows.
            xg_sb = big.tile([128, NG, KCH * 128], BIG_DT)
            w_sb = big.tile([128, KCH, 3 * P], BIG_DT)
            Q = KCH * 128 // 4
            nc.sync.dma_start(out=xg_sb[:, 0, 0:Q], in_=xTg[0][:, 0:Q])
            nc.sync.dma_start(out=w_sb[:, 0, :], in_=wT[0])
            for q in range(1, 4):
                nc.sync.dma_start(out=xg_sb[:, 0, q * Q:(q + 1) * Q],
                                  in_=xTg[0][:, q * Q:(q + 1) * Q])
            for k in range(1, KCH):
                nc.sync.dma_start(out=w_sb[:, k, :], in_=wT[k])
            nc.sync.dma_start(out=xg_sb[:, 1, :], in_=xTg[1])
            nc.sync.dma_start(out=xg_sb[:, 2, :], in_=xTg[2])
            si_sb = big.tile([128, NG, NSEG], F32R)
            nc.sync.dma_start(out=si_sb, in_=segind)
            b12_sb = big.tile([128, P], F32)
            nc.sync.dma_start(out=b12_sb, in_=b12)
            fcT_sb = big.tile([128, 2, P], F32R)
            nc.sync.dma_start(out=fcT_sb, in_=fcT)
            projb_sb = big.tile([NSEG, P], F32)
            nc.sync.dma_start(out=projb_sb, in_=projb)
            fcb_sb = big.tile([NSEG, P], F32)
            nc.sync.dma_start(out=fcb_sb, in_=fcb)
            lng_sb = big.tile([NSEG, P], F32)
            nc.sync.dma_start(out=lng_sb, in_=lng)
            lnb_sb = big.tile([NSEG, P], F32)
            nc.sync.dma_start(out=lnb_sb, in_=lnb)
            id_sb = big.tile([NSEG, NSEG], F32R)
            nc.sync.dma_start(out=id_sb, in_=ident)
            a3_sb = big.tile([128, NPAIR, 128], BIG_DT)
            nc.sync.dma_start(out=a3_sb, in_=a3T)
            for g in range(3, NG):
                nc.sync.dma_start(out=xg_sb[:, g, :], in_=xTg[g])

            x1b_sb = big.tile([128, NG, P], F32)
            x2_sb = big.tile([128, NG, P], BIG_DT)
            psS = pS.tile([NSEG, P], F32)

            # Warm the ACT function tables (Tanh/Copy/Sqrt) during the DMA
            # phase so LoadActFuncSet is off the critical path of the tail.
            warm = tail.tile([1, 1], F32, tag="warm")
            nc.vector.memset(warm, 0.0)
            warm2 = tail.tile([1, 1], F32, tag="warm2")
            nc.scalar.activation(warm2, warm, mybir.ActivationFunctionType.Sqrt,
                                 bias=warm)
            nc.scalar.activation(warm2, warm,
                                 mybir.ActivationFunctionType.Gelu_apprx_tanh)

            def emit_b(t):
                plist = tgt_pairs[t]
                psB = pB.tile([128, P], F32, tag="pB")
                for i, (sg, pi) in enumerate(plist):
                    nc.tensor.matmul(psB, a3_sb[:, pi, :], x2_sb[:, sg, :],
                                     start=(i == 0), stop=(i == len(plist) - 1))
                y1t = y1p.tile([128, P], F32, tag="y1t")
                nc.vector.tensor_add(y1t, x1b_sb[:, t, :], psB)
                nc.sync.dma_start(out=y1[t * 128:(t + 1) * 128, :], in_=y1t)

            # ---- main loop: X123 matmuls + per-group epilogues ----
            for g in range(NG):
                psA = pA.tile([128, 3 * P], F32, tag="pA")
                for k in range(KCH):
                    lhsT = xg_sb[:, g, k * 128:(k + 1) * 128]
                    nc.tensor.matmul(psA[:, 0:512], lhsT, w_sb[:, k, 0:512],
                                     start=(k == 0), stop=(k == KCH - 1))
                    nc.tensor.matmul(psA[:, 512:768], lhsT, w_sb[:, k, 512:768],
                                     start=(k == 0), stop=(k == KCH - 1))
                nc.vector.tensor_add(x1b_sb[:, g, :], b12_sb, psA[:, 0:P])
                nc.vector.tensor_copy(x2_sb[:, g, :], psA[:, P:2 * P])
                x3t = x3p.tile([128, P], F32R, tag="x3t")
                nc.vector.tensor_copy(x3t, psA[:, 2 * P:3 * P])
                nc.tensor.matmul(psS, si_sb[:, g, :], x3t,
                                 start=(g == 0), stop=(g == NG - 1))
                if g >= 2:
                    emit_b(g - 2)

            # ---- projection head on [NSEG, 256] ----
            # DVE/ACT part overlaps with the remaining A'/output matmuls.
            pr = tail.tile([NSEG, P], F32, tag="pr")
            nc.vector.tensor_add(pr, projb_sb, psS)
            t_g = tail.tile([NSEG, P], F32R, tag="t_g")
            nc.scalar.activation(t_g, pr,
                                 mybir.ActivationFunctionType.Gelu_apprx_tanh)

            emit_b(NG - 2)
            emit_b(NG - 1)

            gT = tail.tile([128, 2, NSEG], F32R, tag="gT")
            for c in range(2):
                ptr = pT.tile([128, NSEG], F32R, tag="ptr")
                nc.tensor.transpose(ptr, t_g[:, c * 128:(c + 1) * 128], id_sb)
                nc.vector.tensor_copy(gT[:, c, :], ptr)

            psF = pB.tile([128, P], F32, tag="pB")
            for c in range(2):
                nc.tensor.matmul(psF[0:NSEG, :], gT[:, c, :], fcT_sb[:, c, :],
                                 start=(c == 0), stop=(c == 1))
            t_h0 = tail.tile([NSEG, P], F32, tag="t_h0")
            nc.vector.tensor_add(t_h0, fcb_sb, psF[0:NSEG, :])
            t_h = tail.tile([NSEG, P], F32, tag="t_h")
            nc.vector.tensor_add(t_h, t_h0, pr)

            stats = tail.tile([NSEG, 6], F32, tag="stats")
            nc.vector.bn_stats(out=stats, in_=t_h)
            mv = tail.tile([NSEG, 2], F32, tag="mv")
            nc.vector.bn_aggr(out=mv, in_=stats)
            epst = tail.tile([NSEG, 1], F32, tag="epst")
            nc.vector.memset(epst, LN_EPS)
            sd = tail.tile([NSEG, 1], F32, tag="sd")
            nc.scalar.activation(sd, mv[:, 1:2], mybir.ActivationFunctionType.Sqrt,
                                 bias=epst)
            rstd = tail.tile([NSEG, 1], F32, tag="rstd")
            nc.vector.reciprocal(rstd, sd)
            t_dn = tail.tile([NSEG, P], F32, tag="t_dn")
            nc.vector.tensor_scalar(t_dn, t_h, mv[:, 0:1], rstd,
                                    op0=mybir.AluOpType.subtract,
                                    op1=mybir.AluOpType.mult)
            t_y2g = tail.tile([NSEG, P], F32, tag="t_y2g")
            nc.vector.tensor_mul(t_y2g, t_dn, lng_sb)
            t_y2 = tail.tile([NSEG, P], F32, tag="t_y2")
            nc.vector.tensor_add(t_y2, t_y2g, lnb_sb)
            nc.sync.dma_start(out=y2, in_=t_y2)

    nc.compile()
    return nc


# ----------------------------------------------------------------------------
# entry point
# ----------------------------------------------------------------------------

def kernel(x, label, label_batch, W1_w, W1_b, W2_w, W2_b,
           proj_w, proj_b, fc_w, fc_b, ln_g, ln_b):
    x = np.asarray(x, np.float32)
    label = np.asarray(label)
    label_batch = np.asarray(label_batch)
    N = x.shape[0]

    cores, NG, NSEG = _plan(label, label_batch)
    pairs = ([(g, g) for g in range(NG)]
             + [(g, g + 1) for g in range(NG - 1)]
             + [(g + 1, g) for g in range(NG - 1)])
    NPAIR = len(pairs)
    pair_map = -np.ones((NG, NG), np.int64)
    for pi, (sg, tg) in enumerate(pairs):
        pair_map[sg, tg] = pi

    key = (NG, NSEG, NPAIR)
    if key not in _PROG_CACHE:
        _PROG_CACHE[key] = _build_program(NG, NSEG, NPAIR, pairs)
    nc = _PROG_CACHE[key]

    # replicated weights
    W123T = np.ascontiguousarray(
        np.concatenate([np.asarray(W1_w).T, np.asarray(W2_w).T,
                        np.asarray(proj_w).T], axis=1).astype(np.float32))
    wT_dev = np.ascontiguousarray(W123T.reshape(KCH, 128, 3 * P))
    if BIG_DT == BF16:
        wT_dev = wT_dev.astype(ml_dtypes.bfloat16)
    b12_dev = np.ascontiguousarray(
        np.broadcast_to((np.asarray(W1_b) + np.asarray(W2_b)).astype(np.float32),
                        (128, P)))
    fcT_dev = np.ascontiguousarray(
        np.asarray(fc_w).T.astype(np.float32).reshape(2, 128, P).transpose(1, 0, 2))

    def rep(v):
        return np.ascontiguousarray(
            np.broadcast_to(np.asarray(v).astype(np.float32), (NSEG, P)))

    projb_dev, fcb_dev = rep(proj_b), rep(fc_b)
    lng_dev, lnb_dev = rep(ln_g), rep(ln_b)
    ident_dev = np.eye(NSEG, dtype=np.float32)

    in_maps = []
    packs = []
    for c in range(NCORES):
        xTg, a3T_dev, segind_dev, slots, rows = _pack_core(
            cores[c], x, label, NG, NSEG, pairs, pair_map)
        if BIG_DT == BF16:
            xTg = xTg.astype(ml_dtypes.bfloat16)
            a3T_dev = a3T_dev.astype(ml_dtypes.bfloat16)
        packs.append((slots, rows, cores[c]))
        in_maps.append({
            "xTg": xTg, "wT": wT_dev, "a3T": a3T_dev, "segind": segind_dev,
            "b12": b12_dev, "fcT": fcT_dev, "projb": projb_dev, "fcb": fcb_dev,
            "lng": lng_dev, "lnb": lnb_dev, "ident": ident_dev,
        })

    res = run_bass_kernel_spmd(nc, in_maps, list(range(NCORES)))

    out1 = np.zeros((N, P), np.float32)
    out2 = np.zeros((N, P), np.float32)
    for c in range(NCORES):
        slots, rows, seg_meta = packs[c]
        out1[rows] = res.results[c]["y1"][slots]
        y2c = res.results[c]["y2"]
        for ls, (q0, cnt, r0) in enumerate(seg_meta):
            out2[r0:r0 + cnt] = y2c[ls]
    return out1, out2


# revision 9
# speedup vs baseline: 52098.0554x; 29261.7493x over previous
"""Trainium2 Bass kernel for nn_PrescriptionPill (segment_reduce).

Math (see reference): with xd = x (detached),
  out1[n] = x[n]@W1.T + W1_b + W2_b + loo_mean[n]@W2.T
            where loo_mean is the leave-label-out per-segment mean.
  out2[n] = LN(fc(gelu_tanh(pr)) + fc_b + pr)[seg(n)],  pr = seg_mean@proj.T + proj_b

Everything is linear in x up to the small nonlinear projection head, so:
  X1|X2|X3 = x @ [W1.T | W2.T | proj.T]        (the only big matmuls)
  out1     = X1 + b12 + A' @ X2                (A' block-diagonal per segment,
                                                scaled by 1/other_cnt, built on host
                                                from the integer labels)
  pr       = S @ X3 + proj_b                   (S = per-segment mean indicator)
then the projection head runs on [nseg, 256] only.

Sharding: data-parallel over N, aligned to label_batch segments (each of the
64 prescriptions lives on exactly one of the 8 cores); the small weights are
replicated. Rows are re-packed per core so no segment spans more than two
128-row groups; all gathers/scatters become dense 128x128 indicator matmuls.

Matmuls run as float32r (full-rate PE) accumulating into fp32 PSUM.
"""

import numpy as np
import ml_dtypes
from contextlib import ExitStack

import concourse.bacc as bacc
import concourse.tile as tile
from concourse import mybir
from concourse.bass_utils import run_bass_kernel_spmd

F32 = mybir.dt.float32
F32R = mybir.dt.float32r
BF16 = mybir.dt.bfloat16

# dtype of the two big streamed operands (x and the stacked weights).
# bf16 halves their DMA traffic; PE streams 1 col/cycle either way.
BIG_DT = BF16

D = 2048
P = 256
KCH = D // 128          # 16 contraction chunks
NCORES = 8
GELU_C0 = 0.7978845608028654
GELU_C1 = 0.044715
LN_EPS = 1e-5

_PROG_CACHE = {}


# ----------------------------------------------------------------------------
# host-side planning
# ----------------------------------------------------------------------------

def _plan(label, label_batch):
    """Segment-aligned sharding + per-core row packing."""
    N = label_batch.shape[0]
    segs, seg_starts, seg_cnts = np.unique(label_batch, return_index=True,
                                           return_counts=True)
    nseg = len(segs)
    cum = np.cumsum(seg_cnts)

    bounds = [0]
    for c in range(1, NCORES):
        target = N * c / NCORES
        i = int(np.argmin(np.abs(cum - target))) + 1
        bounds.append(max(i, bounds[-1] + 1))
    bounds.append(nseg)

    cores = []
    maxslots = 0
    maxseg = 0
    for c in range(NCORES):
        s0, s1 = bounds[c], bounds[c + 1]
        q = 0
        seg_meta = []            # (slot0, cnt, orig_row0)
        for s in range(s0, s1):
            cnt = int(seg_cnts[s])
            assert cnt <= 256, "segment larger than two row groups"
            if cnt > 128 and (q % 128) + cnt > 256:
                q = ((q + 127) // 128) * 128
            seg_meta.append((q, cnt, int(seg_starts[s])))
            q += cnt
        cores.append(seg_meta)
        maxslots = max(maxslots, q)
        maxseg = max(maxseg, s1 - s0)

    NG = (maxslots + 127) // 128
    return cores, NG, maxseg


def _pack_core(seg_meta, x, label, NG, NSEG, pairs, pair_map):
    """Build this core's device input tensors."""
    NMAX = NG * 128
    NPAIR = len(pairs)
    slots_list, rows_list = [], []
    for (q0, cnt, r0) in seg_meta:
        slots_list.append(np.arange(q0, q0 + cnt))
        rows_list.append(np.arange(r0, r0 + cnt))
    slots = np.concatenate(slots_list)
    rows = np.concatenate(rows_list)

    xp = np.zeros((NMAX, D), np.float32)
    xp[slots] = x[rows]
    # xTg[g, p, k*128+n] = xp[g*128+n, k*128+p]
    xTg = np.ascontiguousarray(
        xp.reshape(NG, 128, KCH, 128).transpose(0, 3, 2, 1))

    a3T = np.zeros((NPAIR, 128, 128), np.float32)
    segind = np.zeros((128, NG, NSEG), np.float32)
    for ls, (q0, cnt, r0) in enumerate(seg_meta):
        l = label[r0:r0 + cnt]
        same = l[:, None] == l[None, :]
        other_cnt = cnt - same.sum(1)
        coef = np.where(other_cnt > 0, 1.0 / np.maximum(other_cnt, 1), 0.0)
        M = (~same) * coef[None, :].astype(np.float32)
        si = np.arange(q0, q0 + cnt)
        pidx = pair_map[si[:, None] // 128, si[None, :] // 128]
        assert (pidx >= 0).all()
        flat = (pidx * 128 + (si % 128)[:, None]) * 128 + (si % 128)[None, :]
        a3T.reshape(-1)[flat.ravel()] = M.ravel().astype(np.float32)
        segind[si % 128, si // 128, ls] = 1.0 / cnt

    # a3T device layout: [src_r, pair, tgt_r]
    a3T_dev = np.ascontiguousarray(a3T.transpose(1, 0, 2))
    return xTg, a3T_dev, segind, slots, rows


# ----------------------------------------------------------------------------
# device program
# ----------------------------------------------------------------------------

def _build_program(NG, NSEG, NPAIR, pairs):
    nc = bacc.Bacc("TRN2", target_bir_lowering=False, debug=False)
    NMAX = NG * 128

    xTg = nc.dram_tensor("xTg", [NG, 128, KCH * 128], BIG_DT, kind="ExternalInput").ap()
    wT = nc.dram_tensor("wT", [KCH, 128, 3 * P], BIG_DT, kind="ExternalInput").ap()
    a3T = nc.dram_tensor("a3T", [128, NPAIR, 128], BIG_DT, kind="ExternalInput").ap()
    segind = nc.dram_tensor("segind", [128, NG, NSEG], F32R, kind="ExternalInput").ap()
    b12 = nc.dram_tensor("b12", [128, P], F32, kind="ExternalInput").ap()
    fcT = nc.dram_tensor("fcT", [128, 2, P], BIG_DT, kind="ExternalInput").ap()
    projb = nc.dram_tensor("projb", [NSEG, P], F32, kind="ExternalInput").ap()
    fcb = nc.dram_tensor("fcb", [NSEG, P], F32, kind="ExternalInput").ap()
    lng = nc.dram_tensor("lng", [NSEG, P], F32, kind="ExternalInput").ap()
    lnb = nc.dram_tensor("lnb", [NSEG, P], F32, kind="ExternalInput").ap()
    ident = nc.dram_tensor("ident", [NSEG, NSEG], F32, kind="ExternalInput").ap()
    y1 = nc.dram_tensor("y1", [NMAX, P], F32, kind="ExternalOutput").ap()
    y2 = nc.dram_tensor("y2", [NSEG, P], F32, kind="ExternalOutput").ap()

    # pairs with a given target group, as (src_group, pair_index)
    tgt_pairs = {t: [] for t in range(NG)}
    for pi, (sg, tg) in enumerate(pairs):
        tgt_pairs[tg].append((sg, pi))

    with tile.TileContext(nc) as tc:
        with ExitStack() as ctx:
            big = ctx.enter_context(tc.tile_pool(name="big", bufs=1))
            x3p = ctx.enter_context(tc.tile_pool(name="x3p", bufs=3))
            y1p = ctx.enter_context(tc.tile_pool(name="y1p", bufs=3))
            tail = ctx.enter_context(tc.tile_pool(name="tail", bufs=1))
            pA = ctx.enter_context(tc.tile_pool(name="pA", bufs=2, space="PSUM"))
            pB = ctx.enter_context(tc.tile_pool(name="pB", bufs=2, space="PSUM"))
            pS = ctx.enter_context(tc.tile_pool(name="pS", bufs=1, space="PSUM"))
            pT = ctx.enter_context(tc.tile_pool(name="pT", bufs=1, space="PSUM"))

            # ---- input loads ----
            # Order matters: the DMA stream is ~bandwidth-serial, and PE's
            # first group needs xg0 + W chunks in k order. Everything later
            # (xg1.., a3, tail constants) follows.
            xg_sb = big.tile([128, NG, KCH * 128], BIG_DT)
            w_sb = big.tile([128, KCH, 3 * P], BIG_DT)
            Q = KCH * 128 // 4
            for q in range(4):
                nc.sync.dma_start(out=xg_sb[:, 0, q * Q:(q + 1) * Q],
                                  in_=xTg[0][:, q * Q:(q + 1) * Q])
                for k in range(4 * q, 4 * q + 4):
                    nc.sync.dma_start(out=w_sb[:, k, :], in_=wT[k])
            nc.sync.dma_start(out=xg_sb[:, 1, :], in_=xTg[1])
            nc.sync.dma_start(out=xg_sb[:, 2, :], in_=xTg[2])
            si_sb = big.tile([128, NG, NSEG], F32R)
            nc.sync.dma_start(out=si_sb, in_=segind)
            b12_sb = big.tile([128, P], F32)
            nc.sync.dma_start(out=b12_sb, in_=b12)
            fcT_sb = big.tile([128, 2, P], BIG_DT)
            nc.sync.dma_start(out=fcT_sb, in_=fcT)
            projb_sb = big.tile([NSEG, P], F32)
            nc.sync.dma_start(out=projb_sb, in_=projb)
            fcb_sb = big.tile([NSEG, P], F32)
            nc.sync.dma_start(out=fcb_sb, in_=fcb)
            lng_sb = big.tile([NSEG, P], F32)
            nc.sync.dma_start(out=lng_sb, in_=lng)
            lnb_sb = big.tile([NSEG, P], F32)
            nc.sync.dma_start(out=lnb_sb, in_=lnb)
            id_sb = big.tile([NSEG, NSEG], F32)
            nc.sync.dma_start(out=id_sb, in_=ident)
            a3_sb = big.tile([128, NPAIR, 128], BIG_DT)
            nc.sync.dma_start(out=a3_sb, in_=a3T)
            for g in range(3, NG):
                nc.sync.dma_start(out=xg_sb[:, g, :], in_=xTg[g])

            x1b_sb = big.tile([128, NG, P], F32)
            x2_sb = big.tile([128, NG, P], BIG_DT)
            psS = pS.tile([NSEG, P], F32)

            # Warm the ACT function tables (Tanh/Copy/Sqrt) during the DMA
            # phase so LoadActFuncSet is off the critical path of the tail.
            warm = tail.tile([1, 1], F32, tag="warm")
            nc.vector.memset(warm, 0.0)
            warm2 = tail.tile([1, 1], F32, tag="warm2")
            nc.scalar.activation(warm2, warm, mybir.ActivationFunctionType.Sqrt,
                                 bias=warm)
            nc.scalar.activation(warm2, warm,
                                 mybir.ActivationFunctionType.Gelu_apprx_tanh)

            def emit_b(t):
                plist = tgt_pairs[t]
                psB = pB.tile([128, P], F32, tag="pB")
                for i, (sg, pi) in enumerate(plist):
                    nc.tensor.matmul(psB, a3_sb[:, pi, :], x2_sb[:, sg, :],
                                     start=(i == 0), stop=(i == len(plist) - 1))
                y1t = y1p.tile([128, P], F32, tag="y1t")
                nc.vector.tensor_add(y1t, x1b_sb[:, t, :], psB)
                nc.sync.dma_start(out=y1[t * 128:(t + 1) * 128, :], in_=y1t)

            # ---- main loop: X123 matmuls + per-group epilogues ----
            for g in range(NG):
                psA = pA.tile([128, 3 * P], F32, tag="pA")
                for k in range(KCH):
                    lhsT = xg_sb[:, g, k * 128:(k + 1) * 128]
                    nc.tensor.matmul(psA[:, 0:512], lhsT, w_sb[:, k, 0:512],
                                     start=(k == 0), stop=(k == KCH - 1))
                    nc.tensor.matmul(psA[:, 512:768], lhsT, w_sb[:, k, 512:768],
                                     start=(k == 0), stop=(k == KCH - 1))
                nc.vector.tensor_add(x1b_sb[:, g, :], b12_sb, psA[:, 0:P])
                nc.vector.tensor_copy(x2_sb[:, g, :], psA[:, P:2 * P])
                x3t = x3p.tile([128, P], F32R, tag="x3t")
                nc.vector.tensor_copy(x3t, psA[:, 2 * P:3 * P])
                nc.tensor.matmul(psS, si_sb[:, g, :], x3t,
                                 start=(g == 0), stop=(g == NG - 1))
                if g >= 2:
                    emit_b(g - 2)

            # ---- projection head on [NSEG, 256] ----
            # DVE/ACT part overlaps with the remaining A'/output matmuls.
            pr = tail.tile([NSEG, P], F32, tag="pr")
            nc.vector.tensor_add(pr, projb_sb, psS)

            emit_b(NG - 2)

            gT = tail.tile([128, 2, NSEG], BIG_DT, tag="gT")
            for c in range(2):
                ptr = pT.tile([128, NSEG], F32, tag="ptr")
                nc.tensor.transpose(ptr, pr[:, c * 128:(c + 1) * 128], id_sb)
                nc.scalar.activation(gT[:, c, :], ptr,
                                     mybir.ActivationFunctionType.Gelu_apprx_tanh)

            emit_b(NG - 1)

            psF = pB.tile([128, P], F32, tag="pB")
            for c in range(2):
                nc.tensor.matmul(psF[0:NSEG, :], gT[:, c, :], fcT_sb[:, c, :],
                                 start=(c == 0), stop=(c == 1))
            t_h0 = tail.tile([NSEG, P], F32, tag="t_h0")
            nc.vector.tensor_add(t_h0, fcb_sb, psF[0:NSEG, :])
            t_h = tail.tile([NSEG, P], F32, tag="t_h")
            nc.vector.tensor_add(t_h, t_h0, pr)

            stats = tail.tile([NSEG, 6], F32, tag="stats")
            nc.vector.bn_stats(out=stats, in_=t_h)
            mv = tail.tile([NSEG, 2], F32, tag="mv")
            nc.vector.bn_aggr(out=mv, in_=stats)
            epst = tail.tile([NSEG, 1], F32, tag="epst")
            nc.vector.memset(epst, LN_EPS)
            sd = tail.tile([NSEG, 1], F32, tag="sd")
            nc.scalar.activation(sd, mv[:, 1:2], mybir.ActivationFunctionType.Sqrt,
                                 bias=epst)
            rstd = tail.tile([NSEG, 1], F32, tag="rstd")
            nc.vector.reciprocal(rstd, sd)
            t_dn = tail.tile([NSEG, P], F32, tag="t_dn")
            nc.vector.tensor_scalar(t_dn, t_h, mv[:, 0:1], rstd,
                                    op0=mybir.AluOpType.subtract,
                                    op1=mybir.AluOpType.mult)
            t_y2g = tail.tile([NSEG, P], F32, tag="t_y2g")
            nc.vector.tensor_mul(t_y2g, t_dn, lng_sb)
            t_y2 = tail.tile([NSEG, P], F32, tag="t_y2")
            nc.vector.tensor_add(t_y2, t_y2g, lnb_sb)
            nc.sync.dma_start(out=y2, in_=t_y2)

    nc.compile()
    return nc


# ----------------------------------------------------------------------------
# entry point
# ----------------------------------------------------------------------------

def kernel(x, label, label_batch, W1_w, W1_b, W2_w, W2_b,
           proj_w, proj_b, fc_w, fc_b, ln_g, ln_b):
    x = np.asarray(x, np.float32)
    label = np.asarray(label)
    label_batch = np.asarray(label_batch)
    N = x.shape[0]

    cores, NG, NSEG = _plan(label, label_batch)
    pairs = ([(g, g) for g in range(NG)]
             + [(g, g + 1) for g in range(NG - 1)]
             + [(g + 1, g) for g in range(NG - 1)])
    NPAIR = len(pairs)
    pair_map = -np.ones((NG, NG), np.int64)
    for pi, (sg, tg) in enumerate(pairs):
        pair_map[sg, tg] = pi

    key = (NG, NSEG, NPAIR)
    if key not in _PROG_CACHE:
        _PROG_CACHE[key] = _build_program(NG, NSEG, NPAIR, pairs)
    nc = _PROG_CACHE[key]

    # replicated weights
    W123T = np.ascontiguousarray(
        np.concatenate([np.asarray(W1_w).T, np.asarray(W2_w).T,
                        np.asarray(proj_w).T], axis=1).astype(np.float32))
    wT_dev = np.ascontiguousarray(W123T.reshape(KCH, 128, 3 * P))
    if BIG_DT == BF16:
        wT_dev = wT_dev.astype(ml_dtypes.bfloat16)
    b12_dev = np.ascontiguousarray(
        np.broadcast_to((np.asarray(W1_b) + np.asarray(W2_b)).astype(np.float32),
                        (128, P)))
    fcT_dev = np.ascontiguousarray(
        np.asarray(fc_w).T.astype(np.float32).reshape(2, 128, P).transpose(1, 0, 2))
    if BIG_DT == BF16:
        fcT_dev = fcT_dev.astype(ml_dtypes.bfloat16)

    def rep(v):
        return np.ascontiguousarray(
            np.broadcast_to(np.asarray(v).astype(np.float32), (NSEG, P)))

    projb_dev, fcb_dev = rep(proj_b), rep(fc_b)
    lng_dev, lnb_dev = rep(ln_g), rep(ln_b)
    ident_dev = np.eye(NSEG, dtype=np.float32)

    in_maps = []
    packs = []
    for c in range(NCORES):
        xTg, a3T_dev, segind_dev, slots, rows = _pack_core(
            cores[c], x, label, NG, NSEG, pairs, pair_map)
        if BIG_DT == BF16:
            xTg = xTg.astype(ml_dtypes.bfloat16)
            a3T_dev = a3T_dev.astype(ml_dtypes.bfloat16)
        packs.append((slots, rows, cores[c]))
        in_maps.append({
            "xTg": xTg, "wT": wT_dev, "a3T": a3T_dev, "segind": segind_dev,
            "b12": b12_dev, "fcT": fcT_dev, "projb": projb_dev, "fcb": fcb_dev,
            "lng": lng_dev, "lnb": lnb_dev, "ident": ident_dev,
        })

    res = run_bass_kernel_spmd(nc, in_maps, list(range(NCORES)))

    out1 = np.zeros((N, P), np.float32)
    out2 = np.zeros((N, P), np.float32)
    for c in range(NCORES):
        slots, rows, seg_meta = packs[c]
        out1[rows] = res.results[c]["y1"][slots]
        y2c = res.results[c]["y2"]
        for ls, (q0, cnt, r0) in enumerate(seg_meta):
            out2[r0:r0 + cnt] = y2c[ls]
    return out1, out2
